# revision 4
# baseline (speedup 1.0000x reference)
"""Trainium2 Bass kernel for nn_Attention_45303315038988 (v2).

  q = p @ Wh.T (+bh) ; k = r @ Wl.T + bl ; v = p @ Wg.T + bg     [N, D]
  scores = q @ k.T ; attn = softmax(scores, axis=0) ; out = p + attn @ v

Key restructuring vs v1:
  - Host precomputes Wc = Wh.T @ Wl, so scores = p @ Wc @ r.T + p@(Wh.T@bl)
    + per-key constants (which the query-axis softmax cancels).  The device
    therefore AllGathers RAW r (no compute dependency -> collective kicked at
    t=0, in two key-halves) and the k-projection disappears.
  - The per-query bias B_i = p_i@(Wh.T@bl) is folded in exactly by weighting
    E rows: E_w = exp(s - m) * eB (eB = exp(B) host-computed, replicated
    across partitions).  One fused DVE tensor_tensor_reduce does the multiply
    AND the softmax denominator sum.  Since e^{B_i} scales whole E columns,
    phase E output needs no correction.
  - Softmax globalization stats (per-key -max, weighted sum) are stored in
    PROCESSING order so each 32-group half AllGathers right after its sweep;
    the global correction f = exp(m_loc - M)/S is applied to the streamed V
    tiles in phase E (ACT idle there), not to E.
  - gpsimd queue carries ONLY collectives (r0, r1, v, st0, st1 in that
    order); data DMAs ride sync/scalar/vector queues so nothing stalls
    behind a collective.
All matmul operands fp16 with fp32 PSUM accumulation; stats fp32.
"""
import numpy as np

P = 128
D = 1024
N = 8192
NCORES = 8
NL = N // NCORES      # 1024 local rows
DB = D // P           # 8 contraction blocks
NG = N // P           # 64 global key blocks
IB = NL // P          # 8 local query blocks
FH = 512
NL2 = NL // 2         # 512 keys per AG half
JB2 = NL2 // P        # 4 key blocks per half
NH = NG // 2          # 32 groups per stats half


def build_nc(k_iters: int = 1, no_cc: bool = False, phases: str = "full",
             opts: dict | None = None, spin_us: int = 0):
    opts = opts or {}
    import concourse.mybir as mybir
    import concourse.tile as tile
    from concourse import bacc

    f16 = mybir.dt.float16
    f32 = mybir.dt.float32
    AF = mybir.ActivationFunctionType
    AX = mybir.AxisListType
    ALU = mybir.AluOpType
    RG = [list(range(NCORES))]

    nc = bacc.Bacc("TRN2", target_bir_lowering=False, debug=False,
                   num_devices=1 if no_cc else NCORES)

    # phases: "full" | "A0" (proj only, no collectives) | "AC0" (proj+scores,
    # no collectives, garbage kt) | "E0" (proj+attnV, no scores/collectives)
    # — for HW phase-time attribution.
    no_coll = phases in ("A0", "AC0", "E0")

    def collective(kind, op, ins, outs):
        if no_coll:
            return
        if no_cc:
            src_ap, dst_ap = ins[0], outs[0]
            nc.sync.dma_start(out=dst_ap[0] if kind == "AllGather" else dst_ap[:],
                              in_=src_ap[:])
        else:
            nc.gpsimd.collective_compute(kind, op, replica_groups=RG,
                                         ins=[ins[0].opt()], outs=[outs[0].opt()])

    pT_h = nc.dram_tensor("pT", [D, NL], f16, kind="ExternalInput")
    rT_h = [nc.dram_tensor(f"rT{h}", [D, NL2], f16, kind="ExternalInput")
            for h in range(2)]
    pres_h = nc.dram_tensor("pres", [NL, D], f32, kind="ExternalInput")
    Wc_h = nc.dram_tensor("Wc", [D, D], f16, kind="ExternalInput")
    WgT_h = nc.dram_tensor("WgT", [D, D], f16, kind="ExternalInput")
    bg_h = nc.dram_tensor("bg16", [1, D], f16, kind="ExternalInput")
    ones_h = nc.dram_tensor("ones16", [1, P], f16, kind="ExternalInput")
    eB_h = nc.dram_tensor("eBr", [P, NL], f16, kind="ExternalInput")
    out_h = nc.dram_tensor("out", [NL, D], f32, kind="ExternalOutput")

    with tile.TileContext(nc) as tc:
        with tc.tile_pool(name="dram", bufs=1, space="DRAM") as dpool:
            for it in range(k_iters):
                cc_r_in = [dpool.tile([D, NL2], f16, name=f"cc_r_in{it}_{h}")
                           for h in range(2)]
                cc_r_out = [dpool.tile([NCORES, D, NL2], f16,
                                       addr_space="Shared",
                                       name=f"cc_r_out{it}_{h}")
                            for h in range(2)]
                cc_v_in = dpool.tile([NL, D], f16, name=f"cc_v_in{it}")
                cc_v_out = dpool.tile([NCORES, NL, D], f16,
                                      addr_space="Shared", name=f"cc_v_out{it}")
                cc_st_in = [dpool.tile([P, 2 * NH], f32, name=f"cc_st_in{it}_{h}")
                            for h in range(2)]
                cc_st_out = [dpool.tile([NCORES, P, 2 * NH], f32,
                                        addr_space="Shared",
                                        name=f"cc_st_out{it}_{h}")
                             for h in range(2)]

                with tc.tile_pool(name="lp", bufs=1) as lp:
                    tT = lp.tile([P, DB, NL], f16)
                    stats = [lp.tile([P, 2, NH], f32, name=f"stats{h}")
                             for h in range(2)]
                    f_sc = lp.tile([P, NG], f32)
                    eB_sb = lp.tile([P, NL], f16)
                    bg_sb = lp.tile([1, D], f16)
                    ones_sb = lp.tile([1, P], f16)
                    nc.sync.dma_start(out=bg_sb, in_=bg_h.ap())
                    nc.sync.dma_start(out=ones_sb, in_=ones_h.ap())
                    nc.sync.dma_start(out=eB_sb, in_=eB_h.ap())

                    # kick AG(r) halves immediately: stage DRAM->DRAM on the
                    # scalar queue, collectives on gpsimd (nothing else there)
                    for h in range(2):
                        nc.scalar.dma_start(out=cc_r_in[h][:, :],
                                            in_=rT_h[h].ap())
                        collective("AllGather", ALU.bypass,
                                   [cc_r_in[h]], [cc_r_out[h]])

                    # ---------------- phase A: projections ----------------
                    with (
                        tc.tile_pool(name="pw", bufs=1) as pw,
                        tc.tile_pool(name="pst", bufs=3) as pst,
                        tc.tile_pool(name="psA", bufs=3, space="PSUM") as psA,
                    ):
                        Wc_sb = pw.tile([P, DB, D], f16)
                        WgT_sb = pw.tile([P, DB, D], f16)
                        pT_sb = pw.tile([P, DB, NL], f16)
                        for db in range(DB):
                            for t_sb, t_h in ((Wc_sb, Wc_h), (pT_sb, pT_h)):
                                nc.sync.dma_start(
                                    out=t_sb[:, db, :],
                                    in_=t_h.ap()[db * P:(db + 1) * P, :])
                        for db in range(DB):
                            nc.sync.dma_start(
                                out=WgT_sb[:, db, :],
                                in_=WgT_h.ap()[db * P:(db + 1) * P, :])

                        # tT = Wc.T @ pT -> [do, i]; stays in SBUF
                        for dob in range(DB):
                            ps_t = psA.tile([P, NL], f32)
                            for db in range(DB):
                                for ih in range(2):
                                    nc.tensor.matmul(
                                        ps_t[:, ih * FH:(ih + 1) * FH],
                                        lhsT=Wc_sb[:, db, dob * P:(dob + 1) * P],
                                        rhs=pT_sb[:, db, ih * FH:(ih + 1) * FH],
                                        start=(db == 0), stop=(db == DB - 1))
                            nc.scalar.activation(out=tT[:, dob, :], in_=ps_t,
                                                 func=AF.Copy)

                        # v shard = pT.T @ WgT -> [j_l, dv], + bg via ones-row
                        for jb in range(IB):
                            ps_t = psA.tile([P, NL], f32)
                            for db in range(DB):
                                for dvh in range(2):
                                    nc.tensor.matmul(
                                        ps_t[:, dvh * FH:(dvh + 1) * FH],
                                        lhsT=pT_sb[:, db, jb * P:(jb + 1) * P],
                                        rhs=WgT_sb[:, db, dvh * FH:(dvh + 1) * FH],
                                        start=(db == 0), stop=False)
                            for dvh in range(2):
                                nc.tensor.matmul(
                                    ps_t[:, dvh * FH:(dvh + 1) * FH],
                                    lhsT=ones_sb[:, :],
                                    rhs=bg_sb[:, dvh * FH:(dvh + 1) * FH],
                                    start=False, stop=True)
                            st = pst.tile([P, NL], f16, tag="st")
                            nc.scalar.activation(out=st, in_=ps_t, func=AF.Copy)
                            # scalar queue: keeps sync free for kt prefetch
                            nc.scalar.dma_start(out=cc_v_in[jb * P:(jb + 1) * P, :],
                                                in_=st)
                        collective("AllGather", ALU.bypass, [cc_v_in], [cc_v_out])

                    if phases == "A0":
                        with tc.tile_pool(name="probe", bufs=2) as prb:
                            pe_t = prb.tile([P, NL], f32, tag="pe")
                            nc.vector.tensor_copy(out=pe_t, in_=tT[:, 0, :])
                            nc.sync.dma_start(out=out_h.ap()[0:P, :], in_=pe_t)
                        continue

                    # -------- phase C: scores^T + weighted local stats -----
                    n_half = 0 if phases == "E0" else 2
                    ep_cm = tc.tile_pool(name="ep", bufs=1)
                    ep = ep_cm.__enter__()
                    E = ep.tile([P, NG, NL], f16)
                    with (
                        tc.tile_pool(name="ktp", bufs=2) as ktp,
                        tc.tile_pool(name="psC", bufs=4, space="PSUM") as psC,
                    ):
                        for h in range(n_half):
                            for c_idx in range(NCORES):
                                kt_c = ktp.tile([P, DB, JB2, P], f16, tag="kt")
                                for db in range(DB):
                                    nc.sync.dma_start(
                                        out=kt_c[:, db, :, :].rearrange(
                                            "p a b -> p (a b)"),
                                        in_=cc_r_out[h][c_idx,
                                                        db * P:(db + 1) * P, :])
                                for jlb in range(JB2):
                                    oc = c_idx * JB2 + jlb
                                    o = h * NH + oc
                                    ps_t = psC.tile([P, NL], f32, tag="sc")
                                    for db in range(DB):
                                        for ih in range(2):
                                            nc.tensor.matmul(
                                                ps_t[:, ih * FH:(ih + 1) * FH],
                                                lhsT=kt_c[:, db, jlb, :],
                                                rhs=tT[:, db, ih * FH:(ih + 1) * FH],
                                                start=(db == 0),
                                                stop=(db == DB - 1))
                                    nc.vector.tensor_reduce(
                                        out=stats[h][:, 0, oc:oc + 1], in_=ps_t,
                                        op=ALU.max, axis=AX.X, negate=True)
                                    nc.scalar.activation(
                                        out=E[:, o, :], in_=ps_t, func=AF.Exp,
                                        bias=stats[h][:, 0, oc:oc + 1], scale=1.0)
                                    # (tensor_tensor_reduce would fuse these
                                    # two, but it crashes the HW ucode)
                                    nc.vector.tensor_mul(
                                        out=E[:, o, :], in0=E[:, o, :],
                                        in1=eB_sb)
                                    nc.vector.tensor_reduce(
                                        out=stats[h][:, 1, oc:oc + 1],
                                        in_=E[:, o, :], op=ALU.add, axis=AX.X)
                            # this half's stats are final: gather them now
                            # (scalar queue: doesn't block the kt load stream)
                            nc.scalar.dma_start(
                                out=cc_st_in[h][:, :],
                                in_=stats[h].rearrange("p a b -> p (a b)"))
                            collective("AllGather", ALU.bypass,
                                       [cc_st_in[h]], [cc_st_out[h]])

                    if phases == "AC0":
                        with tc.tile_pool(name="probe", bufs=2) as prb:
                            pe_t = prb.tile([P, NL], f32, tag="pe")
                            nc.vector.tensor_copy(out=pe_t, in_=E[:, NG - 1, :])
                            nc.sync.dma_start(out=out_h.ap()[0:P, :], in_=pe_t)
                        ep_cm.__exit__(None, None, None)
                        continue

                    # combine halves -> f = exp(m_loc - M) / S_glob  per key
                    Mneg = lp.tile([P, NG], f32)
                    Ssum = lp.tile([P, NG], f32)
                    tmp = lp.tile([P, NG], f32)
                    diff = lp.tile([P, NG], f32)
                    alpha = lp.tile([P, NG], f32)
                    rec = lp.tile([P, NG], f32)
                    gath = [lp.tile([P, NCORES, 2, NH], f32, name=f"gath{h}")
                            for h in range(2)]
                    for h in range(n_half):
                        hs = slice(h * NH, (h + 1) * NH)
                        nc.sync.dma_start(
                            out=gath[h].rearrange("p c a b -> p (c a b)"),
                            in_=cc_st_out[h].rearrange("c p x -> p c x"))
                        g_h = gath[h]
                        nc.vector.tensor_copy(out=Mneg[:, hs], in_=g_h[:, 0, 0, :])
                        for c in range(1, NCORES):
                            nc.vector.tensor_tensor(out=Mneg[:, hs],
                                                    in0=Mneg[:, hs],
                                                    in1=g_h[:, c, 0, :],
                                                    op=ALU.min)
                        for c in range(NCORES):
                            nc.vector.tensor_sub(out=tmp[:, hs],
                                                 in0=Mneg[:, hs],
                                                 in1=g_h[:, c, 0, :])
                            nc.scalar.activation(out=tmp[:, hs],
                                                 in_=tmp[:, hs], func=AF.Exp)
                            nc.vector.tensor_mul(out=tmp[:, hs],
                                                 in0=tmp[:, hs],
                                                 in1=g_h[:, c, 1, :])
                            if c == 0:
                                nc.vector.tensor_copy(out=Ssum[:, hs],
                                                      in_=tmp[:, hs])
                            else:
                                nc.vector.tensor_add(out=Ssum[:, hs],
                                                     in0=Ssum[:, hs],
                                                     in1=tmp[:, hs])
                        nc.vector.tensor_sub(out=diff[:, hs], in0=Mneg[:, hs],
                                             in1=stats[h][:, 0, :])
                        nc.scalar.activation(out=alpha[:, hs], in_=diff[:, hs],
                                             func=AF.Exp)
                        nc.vector.reciprocal(out=rec[:, hs], in_=Ssum[:, hs])
                        nc.vector.tensor_mul(out=f_sc[:, hs], in0=alpha[:, hs],
                                             in1=rec[:, hs])
                        # fold f into E rows now (in place): half-0 scales
                        # hide under phase C's second sweep, and phase E's
                        # inner loop loses the ACT middleman on V tiles
                        for oc in range(NH):
                            o = h * NH + oc
                            nc.scalar.activation(
                                out=E[:, o, :], in_=E[:, o, :], func=AF.Copy,
                                scale=f_sc[:, o:o + 1])

                    if phases == "E0":
                        # bench-only: give f_sc/E a writer (gpsimd casts)
                        nc.gpsimd.dma_start(out=f_sc, in_=eB_h.ap()[:, 0:NG])
                        nc.scalar.activation(out=E[:, 0, :], in_=eB_sb,
                                             func=AF.Copy)

                    # -------- phase E: out = E_w^T.T @ (f*V) + p ----------
                    # two passes over the query axis; each E-block weight
                    # load feeds 2 matmuls (LDWEIGHTS is unmodeled by the
                    # scheduler but real on HW — keep LDW:MM at 1:2).
                    with (
                        tc.tile_pool(name="vtp", bufs=12) as vtp,
                        tc.tile_pool(name="prp", bufs=2) as prp,
                        tc.tile_pool(name="osp", bufs=2) as osp,
                        tc.tile_pool(name="psE", bufs=1, space="PSUM") as psE,
                    ):
                        for ihalf in range(2):
                            po = [psE.tile([P, D], f32, tag=f"po{q_}",
                                           name=f"po{q_}")
                                  for q_ in range(IB // 2)]
                            for o in range(NG):
                                h_o, oc = divmod(o, NH)
                                c_idx, jlb = divmod(oc, JB2)
                                jb = h_o * JB2 + jlb
                                vt = vtp.tile([P, D], f16, tag="vt")
                                # alternate HWDGE rings (scalar idle here) so
                                # V streaming never chains on one ring
                                vq = nc.sync if o % 2 == 0 else nc.scalar
                                vq.dma_start(
                                    out=vt,
                                    in_=cc_v_out[c_idx, jb * P:(jb + 1) * P, :])
                                for q_ in range(IB // 2):
                                    ib = ihalf * (IB // 2) + q_
                                    for dvh in range(2):
                                        nc.tensor.matmul(
                                            po[q_][:, dvh * FH:(dvh + 1) * FH],
                                            lhsT=E[:, o, ib * P:(ib + 1) * P],
                                            rhs=vt[:, dvh * FH:(dvh + 1) * FH],
                                            start=(o == 0), stop=(o == NG - 1))
                            for q_ in range(IB // 2):
                                ib = ihalf * (IB // 2) + q_
                                pr = prp.tile([P, D], f32, tag="pr")
                                nc.scalar.dma_start(
                                    out=pr,
                                    in_=pres_h.ap()[ib * P:(ib + 1) * P, :])
                                ot = osp.tile([P, D], f32, tag="ot")
                                nc.vector.tensor_add(out=ot, in0=po[q_], in1=pr)
                                nc.sync.dma_start(
                                    out=out_h.ap()[ib * P:(ib + 1) * P, :],
                                    in_=ot)
                    ep_cm.__exit__(None, None, None)
            if spin_us:
                with tc.tile_critical():
                    for _ in range(spin_us):
                        nc.vector.nop(cycle_cnt=960)
    nc.compile()
    return nc


def prepare_in_maps(p, r, Wh, bh, Wl, bl, Wg, bg):
    f16 = np.float16
    f32 = np.float32
    Wc = np.ascontiguousarray(Wh.astype(f32).T @ Wl.astype(f32)).astype(f16)
    WgT = np.ascontiguousarray(Wg.T).astype(f16)
    bg16 = bg.astype(f16).reshape(1, D)
    u = Wh.astype(f32).T @ bl.astype(f32)
    in_maps = []
    for c in range(NCORES):
        sl = slice(c * NL, (c + 1) * NL)
        rT = np.ascontiguousarray(r[sl].T).astype(f16)
        eB = np.exp(p[sl].astype(f32) @ u).astype(f16)
        in_maps.append({
            "pT": np.ascontiguousarray(p[sl].T).astype(f16),
            "rT0": np.ascontiguousarray(rT[:, 0:NL2]),
            "rT1": np.ascontiguousarray(rT[:, NL2:NL]),
            "pres": np.ascontiguousarray(p[sl]).astype(f32),
            "Wc": Wc, "WgT": WgT,
            "bg16": bg16, "ones16": np.ones((1, P), f16),
            "eBr": np.ascontiguousarray(
                np.broadcast_to(eB[None, :], (P, NL))),
        })
    return in_maps


_NC_CACHE = {}


def kernel(p, r, Wh, bh, Wl, bl, Wg, bg):
    from concourse.bass_utils import run_bass_kernel_spmd

    p = np.asarray(p); r = np.asarray(r)
    in_maps = prepare_in_maps(p, r, np.asarray(Wh), np.asarray(bh),
                              np.asarray(Wl), np.asarray(bl),
                              np.asarray(Wg), np.asarray(bg))
    if 1 not in _NC_CACHE:
        _NC_CACHE[1] = build_nc(1)
    res = run_bass_kernel_spmd(_NC_CACHE[1], in_maps, list(range(NCORES)))
    out = np.concatenate([res.results[c]["out"] for c in range(NCORES)], axis=0)
    return out.astype(np.float32)
